# revision 1
# baseline (speedup 1.0000x reference)
"""Trainium2 Bass kernel for the 2-layer GAT + mean-pool + log_softmax problem.

Sharding: edges sorted by dst; dst space padded to 100352 nodes and split into
8 contiguous ranges (12544 nodes/core) -> attention softmax + scatter are
fully core-local. Node tables replicated; layer-2 table AllGathered; pooled
sums AllReduced.

Per core: node stage h=[x@W1 | x@W1@A1s | x@W1@A1d] via PE; edge stage per
128-node window: [128,1]-indirect-DMA row gathers by edge src; dst one-hots
via iota is_equal (both orientations; transposed via PE rank-1 broadcast);
attention exp/leaky-relu on ACT/DVE; segment softmax-sums via one-hot scatter
matmul in PSUM; epilogue normalizes + ELU. Layer 2 mirrors with 18-wide rows;
per-graph mean pooling via one-hot matmul vs batch ids; log_softmax on chip.
"""
import numpy as np
from contextlib import ExitStack

import concourse.bass as bass
import concourse.tile as tile
import concourse.mybir as mybir
from concourse import bacc
from concourse.tile import add_dep_helper

F32 = mybir.dt.float32
I32 = mybir.dt.int32
AX = mybir.AxisListType
AF = mybir.ActivationFunctionType
OP = mybir.AluOpType

N = 100000
G = 64
FIN = 128
HID = 16
H = 8
FOUT = 16
NEG = 0.2
NCORES = 8
NP = 100352            # padded nodes (784 * 128)
NPC = NP // NCORES     # nodes per core
WN = 128               # window nodes
WPC = NPC // WN        # windows per core
S = 4                  # tiles per slab
PADLOC = 10000.0

D1 = 144               # T1 row: h(128) | as(8) | ad(8)
D2 = 18                # T2 row: h2(16) | as2(1) | ad2(1)


def _prep(x, edge_index, batch, W1, att1_src, att1_dst, b1, W2, att2_src, att2_dst, b2):
    src = np.asarray(edge_index[0], np.int64)
    dst = np.asarray(edge_index[1], np.int64)
    order = np.argsort(dst, kind="stable")
    src = src[order]
    dst = dst[order]

    nwin = NP // WN
    win = (dst // WN).astype(np.int64)
    wcnt = np.bincount(win, minlength=nwin).reshape(NCORES, WPC)
    ntiles = np.maximum(1, -(-wcnt // 128))
    tiles_per_lw = ntiles.max(axis=0)
    T = int(tiles_per_lw.sum())

    gidx = np.zeros((NCORES, 128, T), np.int32)
    dstloc = np.full((NCORES, 128, T), PADLOC, np.float32)
    dstrow = np.full((NCORES, 1, T * 128), PADLOC, np.float32)
    wstart = np.zeros(nwin + 1, np.int64)
    np.cumsum(np.bincount(win, minlength=nwin), out=wstart[1:])
    tile_base = np.zeros(WPC + 1, np.int64)
    np.cumsum(tiles_per_lw, out=tile_base[1:])
    for c in range(NCORES):
        for w in range(WPC):
            gw = c * WPC + w
            lo, hi = wstart[gw], wstart[gw + 1]
            cnt = hi - lo
            t0 = int(tile_base[w])
            nt = int(tiles_per_lw[w])
            buf_s = np.zeros(nt * 128, np.int32)
            buf_d = np.full(nt * 128, PADLOC, np.float32)
            buf_s[:cnt] = src[lo:hi]
            buf_d[:cnt] = (dst[lo:hi] - gw * WN).astype(np.float32)
            gidx[c, :, t0:t0 + nt] = buf_s.reshape(nt, 128).T
            dstloc[c, :, t0:t0 + nt] = buf_d.reshape(nt, 128).T
            dstrow[c, 0, t0 * 128:(t0 + nt) * 128] = buf_d

    adwidx = np.zeros((NCORES, 128, WPC), np.int32)
    batchloc = np.full((NCORES, 128, WPC), PADLOC, np.float32)
    bpad = np.full(NP, PADLOC, np.float32)
    bpad[:N] = batch.astype(np.float32)
    for c in range(NCORES):
        base = c * NPC
        adwidx[c] = np.arange(base, base + NPC, dtype=np.int32).reshape(WPC, 128).T
        batchloc[c] = bpad[base:base + NPC].reshape(WPC, 128).T

    A1s = np.zeros((FIN, H), np.float32)
    A1d = np.zeros((FIN, H), np.float32)
    for h in range(H):
        A1s[h * HID:(h + 1) * HID, h] = att1_src[h]
        A1d[h * HID:(h + 1) * HID, h] = att1_dst[h]
    W1cat = np.concatenate([W1, W1 @ A1s, W1 @ A1d], axis=1).astype(np.float32)
    W2cat = np.concatenate([W2, W2 @ att2_src[0][:, None], W2 @ att2_dst[0][:, None]],
                           axis=1).astype(np.float32)

    xT = np.zeros((FIN, NP), np.float32)
    xT[:, :N] = np.asarray(x, np.float32).T
    counts = np.bincount(np.asarray(batch, np.int64), minlength=G).astype(np.float32)
    consts = {
        "w1cat": W1cat, "w2cat": W2cat, "xT": xT,
        "b1rep": np.tile(np.asarray(b1, np.float32)[None, :], (128, 1)),
        "b2rep": np.tile(np.asarray(b2, np.float32)[None, :], (G, 1)),
        "cntr": (1.0 / np.maximum(counts, 1.0))[:, None].astype(np.float32),
        "iorow": np.tile(np.arange(128, dtype=np.float32)[None, :], (128, 1)),
        "iocol": np.arange(128, dtype=np.float32)[:, None].copy(),
        "onesr": np.ones((1, 128), np.float32),
        "ident": np.eye(128, dtype=np.float32),
    }
    percore = [{"gidx": gidx[c], "dstloc": dstloc[c], "dstrow": dstrow[c],
                "adwidx": adwidx[c], "batchloc": batchloc[c]} for c in range(NCORES)]
    return consts, percore, tiles_per_lw, tile_base, T


def _build(tiles_per_lw, tile_base, T):
    nc = bacc.Bacc("TRN2", target_bir_lowering=False, debug=False)
    NT1 = NP // 128
    MT = int(max(tiles_per_lw))

    p_xT = nc.declare_dram_parameter("xT", [128, NP], F32, isOutput=False)
    p_w1 = nc.declare_dram_parameter("w1cat", [128, D1], F32, isOutput=False)
    p_w2 = nc.declare_dram_parameter("w2cat", [128, D2], F32, isOutput=False)
    p_b1 = nc.declare_dram_parameter("b1rep", [128, 128], F32, isOutput=False)
    p_b2 = nc.declare_dram_parameter("b2rep", [G, FOUT], F32, isOutput=False)
    p_cn = nc.declare_dram_parameter("cntr", [G, 1], F32, isOutput=False)
    p_ior = nc.declare_dram_parameter("iorow", [128, 128], F32, isOutput=False)
    p_ioc = nc.declare_dram_parameter("iocol", [128, 1], F32, isOutput=False)
    p_on = nc.declare_dram_parameter("onesr", [1, 128], F32, isOutput=False)
    p_id = nc.declare_dram_parameter("ident", [128, 128], F32, isOutput=False)
    p_gi = nc.declare_dram_parameter("gidx", [128, T], I32, isOutput=False)
    p_dl = nc.declare_dram_parameter("dstloc", [128, T], F32, isOutput=False)
    p_dr = nc.declare_dram_parameter("dstrow", [1, T * 128], F32, isOutput=False)
    p_aw = nc.declare_dram_parameter("adwidx", [128, WPC], I32, isOutput=False)
    p_bl = nc.declare_dram_parameter("batchloc", [128, WPC], F32, isOutput=False)
    p_out = nc.declare_dram_parameter("out", [G, FOUT], F32, isOutput=True)

    t1 = nc.dram_tensor("t1tab", [NP, D1], F32)
    t2loc = nc.dram_tensor("t2loc", [NPC, D2], F32)
    t2full = nc.dram_tensor("t2full", [NP, D2], F32, addr_space="Shared")
    prloc = nc.dram_tensor("prloc", [G, FOUT], F32)
    prsum = nc.dram_tensor("prsum", [G, FOUT], F32, addr_space="Shared")

    with ExitStack() as ctx:
        tc = ctx.enter_context(tile.TileContext(nc))
        cst = ctx.enter_context(tc.tile_pool(name="cst", bufs=1))
        big = ctx.enter_context(tc.tile_pool(name="big", bufs=1))
        nod = ctx.enter_context(tc.tile_pool(name="nod", bufs=3))
        pay = ctx.enter_context(tc.tile_pool(name="pay", bufs=6))
        wrk = ctx.enter_context(tc.tile_pool(name="wrk", bufs=3))
        drp = ctx.enter_context(tc.tile_pool(name="drp", bufs=2))
        psA = ctx.enter_context(tc.tile_pool(name="psA", bufs=2, space="PSUM"))
        psB = ctx.enter_context(tc.tile_pool(name="psB", bufs=1, space="PSUM"))
        psC = ctx.enter_context(tc.tile_pool(name="psC", bufs=1, space="PSUM"))
        psW = ctx.enter_context(tc.tile_pool(name="psW", bufs=2, space="PSUM"))
        psP = ctx.enter_context(tc.tile_pool(name="psP", bufs=1, space="PSUM"))

        def ld(pool, shape, dt, src, tag):
            t = pool.tile(shape, dt, tag=tag)
            nc.sync.dma_start(t[:], src)
            return t

        w1c = ld(cst, [128, D1], F32, p_w1[:, :], "w1c")
        w2c = ld(cst, [128, D2], F32, p_w2[:, :], "w2c")
        b1r = ld(cst, [128, 128], F32, p_b1[:, :], "b1r")
        b2r = ld(cst, [G, FOUT], F32, p_b2[:, :], "b2r")
        cnr = ld(cst, [G, 1], F32, p_cn[:, :], "cnr")
        ior = ld(cst, [128, 128], F32, p_ior[:, :], "ior")
        ioc = ld(cst, [128, 1], F32, p_ioc[:, :], "ioc")
        onr = ld(cst, [1, 128], F32, p_on[:, :], "onr")
        idn = ld(cst, [128, 128], F32, p_id[:, :], "idn")
        gix = ld(big, [128, T], I32, p_gi[:, :], "gix")
        dlx = ld(big, [128, T], F32, p_dl[:, :], "dlx")
        awx = ld(big, [128, WPC], I32, p_aw[:, :], "awx")
        blx = ld(big, [128, WPC], F32, p_bl[:, :], "blx")
        h1T = big.tile([128, NPC], F32, tag="h1T")

        # ---------- node stage ----------
        NB = 4
        for bt in range(0, NT1, NB):
            xt = nod.tile([128, NB * 128], F32, tag="xt")
            nc.sync.dma_start(xt[:], p_xT[:, bt * 128:(bt + NB) * 128])
            stg = nod.tile([128, NB * D1], F32, tag="stg")
            for j in range(NB):
                ps = psA.tile([128, D1], F32, space="PSUM", tag="ps_scr")
                nc.tensor.matmul(ps[:], lhsT=xt[:, j * 128:(j + 1) * 128], rhs=w1c[:],
                                 start=True, stop=True)
                if j % 2 == 0:
                    nc.vector.tensor_copy(stg[:, j * D1:(j + 1) * D1], ps[:])
                else:
                    nc.scalar.activation(stg[:, j * D1:(j + 1) * D1], ps[:], AF.Copy)
            nc.sync.dma_start(
                t1[bt * 128:(bt + NB) * 128, :].rearrange("(j p) d -> p j d", p=128),
                stg[:, :].rearrange("p (j d) -> p j d", j=NB))

        def edge_layer(tab_ap, DD, DM, adw_col, adw_elems, epilogue, lname):
            HH = adw_elems
            DC = DM + HH
            CH = DM // HH
            adwall = big.tile([128, WPC * HH], F32, tag="adw" + lname)
            for w in range(WPC):
                nc.gpsimd.indirect_dma_start(
                    out=adwall[:, w * HH:(w + 1) * HH],
                    out_offset=None, in_=tab_ap,
                    in_offset=bass.IndirectOffsetOnAxis(ap=awx[:, w:w + 1], axis=0),
                    element_offset=adw_col)
            gathers = []
            consumers = []
            for w in range(WPC):
                nt = int(tiles_per_lw[w])
                t0 = int(tile_base[w])
                wps = psW.tile([128, DC], F32, space="PSUM", tag="wps")
                adw = adwall[:, w * HH:(w + 1) * HH]
                drw = drp.tile([1, MT * 128], F32, tag="drw")
                nc.sync.dma_start(drw[0:1, :nt * 128],
                                  p_dr[:, t0 * 128:(t0 + nt) * 128])
                for s0 in range(0, nt, S):
                    sn = min(S, nt - s0)
                    tg = t0 + s0
                    pys = pay.tile([128, S * DD], F32, tag="pys" + lname)
                    for j in range(sn):
                        gi = nc.gpsimd.indirect_dma_start(
                            out=pys[:, j * DD:(j + 1) * DD], out_offset=None,
                            in_=tab_ap,
                            in_offset=bass.IndirectOffsetOnAxis(
                                ap=gix[:, tg + j:tg + j + 1], axis=0))
                        gathers.append(gi)
                    dbc = psB.tile([128, S * 128], F32, space="PSUM", tag="dbc")
                    nc.tensor.matmul(dbc[:, :sn * 128], lhsT=onr[:],
                                     rhs=drw[0:1, s0 * 128:(s0 + sn) * 128],
                                     start=True, stop=True)
                    ohne = wrk.tile([128, S * 128], F32, tag="ohne")
                    nc.vector.tensor_tensor(out=ohne[:, :sn * 128],
                                            in0=ioc[:, :].to_broadcast([128, sn * 128]),
                                            in1=dbc[:, :sn * 128], op=OP.is_equal)
                    aps = psC.tile([128, S * HH], F32, space="PSUM", tag="aps")
                    for j in range(sn):
                        nc.tensor.matmul(aps[:, j * HH:(j + 1) * HH],
                                         lhsT=ohne[:, j * 128:(j + 1) * 128],
                                         rhs=adw, start=True, stop=True)
                    pv = pys[:, :].rearrange("p (j d) -> p j d", j=S)
                    sc = wrk.tile([128, S * HH], F32, tag="sc")
                    scv = sc[:, :].rearrange("p (j h) -> p j h", j=S)
                    addi = nc.vector.tensor_tensor(
                        out=scv[:, 0:sn, :],
                        in0=pv[:, 0:sn, DM:DM + HH],
                        in1=aps[:, :].rearrange("p (j h) -> p j h", j=S)[:, 0:sn, :],
                        op=OP.add)
                    consumers.append((addi, len(gathers) - 1))
                    sc2 = wrk.tile([128, S * HH], F32, tag="sc2")
                    nc.vector.tensor_scalar_mul(sc2[:, :sn * HH], sc[:, :sn * HH], NEG)
                    nc.vector.tensor_tensor(out=sc[:, :sn * HH], in0=sc[:, :sn * HH],
                                            in1=sc2[:, :sn * HH], op=OP.max)
                    comb = wrk.tile([128, S * DC], F32, tag="comb" + lname)
                    cv = comb[:, :].rearrange("p (j d) -> p j d", j=S)
                    nc.scalar.activation(
                        cv[:, 0:sn, DM:DM + HH],
                        scv[:, 0:sn, :], AF.Exp)
                    nc.vector.tensor_tensor(
                        out=cv[:, 0:sn, 0:DM].rearrange("p j (h c) -> p j h c", h=HH),
                        in0=pv[:, 0:sn, 0:DM].rearrange("p j (h c) -> p j h c", h=HH),
                        in1=cv[:, 0:sn, DM:DM + HH].unsqueeze(-1
                                                              ).broadcast_to([128, sn, HH, CH]),
                        op=OP.mult)
                    ohen = wrk.tile([128, S * 128], F32, tag="ohen")
                    nc.vector.tensor_tensor(
                        out=ohen[:, :].rearrange("p (j n) -> p j n", j=S)[:, 0:sn, :],
                        in0=dlx[:, tg:tg + sn].unsqueeze(-1).broadcast_to([128, sn, 128]),
                        in1=ior[:, :].unsqueeze(1).broadcast_to([128, sn, 128]),
                        op=OP.is_equal)
                    for j in range(sn):
                        nc.tensor.matmul(wps[:], lhsT=ohen[:, j * 128:(j + 1) * 128],
                                         rhs=comb[:, j * DC:(j + 1) * DC],
                                         start=(s0 == 0 and j == 0),
                                         stop=(s0 + S >= nt and j == sn - 1))
                epilogue(w, wps)
            for (ci, gpos) in consumers:
                tgt = min(gpos + 2 * S, len(gathers) - 1)
                add_dep_helper(ci.ins, gathers[tgt].ins, sync=True,
                               reason="indirect-dma completion skew")

        def epi1(w, wps):
            rec = wrk.tile([128, H], F32, tag="rec")
            nc.vector.tensor_scalar_add(rec[:], wps[:, 128:136], 1e-16)
            nc.vector.reciprocal(rec[:], rec[:])
            o1 = wrk.tile([128, 128], F32, tag="o1")
            nc.vector.tensor_tensor(
                out=o1[:, :].rearrange("p (h c) -> p h c", h=H),
                in0=wps[:, 0:128].rearrange("p (h c) -> p h c", h=H),
                in1=rec[:, :].unsqueeze(-1).broadcast_to([128, H, HID]),
                op=OP.mult)
            nc.vector.tensor_tensor(out=o1[:], in0=o1[:], in1=b1r[:], op=OP.add)
            mn = wrk.tile([128, 128], F32, tag="mn")
            nc.vector.tensor_scalar_min(mn[:], o1[:], 0.0)
            nc.scalar.activation(mn[:], mn[:], AF.Exp)
            nc.vector.tensor_scalar_max(o1[:], o1[:], 0.0)
            nc.vector.tensor_tensor(out=o1[:], in0=o1[:], in1=mn[:], op=OP.add)
            nc.vector.tensor_scalar_add(o1[:], o1[:], -1.0)
            tp = psA.tile([128, 128], F32, space="PSUM", tag="ps_scr")
            nc.tensor.transpose(tp[:], o1[:], idn[:])
            nc.vector.tensor_copy(h1T[:, w * 128:(w + 1) * 128], tp[:])

        edge_layer(t1[:, :], D1, 128, 136, H, epi1, "a")

        # ---------- layer-2 node stage + allgather ----------
        for w0 in range(0, WPC, 4):
            wn = min(4, WPC - w0)
            stg2 = nod.tile([128, 4 * D2], F32, tag="stg2")
            for j in range(wn):
                ps = psA.tile([128, D2], F32, space="PSUM", tag="ps_scr")
                nc.tensor.matmul(ps[:], lhsT=h1T[:, (w0 + j) * 128:(w0 + j + 1) * 128],
                                 rhs=w2c[:], start=True, stop=True)
                nc.vector.tensor_copy(stg2[:, j * D2:(j + 1) * D2], ps[:])
            nc.sync.dma_start(
                t2loc[w0 * 128:(w0 + wn) * 128, :].rearrange("(j p) d -> p j d", p=128),
                stg2[:, :].rearrange("p (j d) -> p j d", j=4)[:, 0:wn, :])
        nc.gpsimd.collective_compute(
            "AllGather", OP.bypass,
            replica_groups=[list(range(NCORES))],
            ins=[t2loc[:, :]], outs=[t2full[:, :]])

        pool_ps = psP.tile([G, FOUT], F32, space="PSUM", tag="pool")

        def epi2(w, wps):
            rec = wrk.tile([128, 1], F32, tag="rec2")
            nc.vector.tensor_scalar_add(rec[:], wps[:, 16:17], 1e-16)
            nc.vector.reciprocal(rec[:], rec[:])
            o2 = wrk.tile([128, FOUT], F32, tag="o2")
            nc.vector.tensor_tensor(out=o2[:], in0=wps[:, 0:16],
                                    in1=rec[:, :].to_broadcast([128, FOUT]), op=OP.mult)
            og = wrk.tile([128, G], F32, tag="og")
            nc.vector.tensor_tensor(out=og[:], in0=blx[:, w:w + 1].to_broadcast([128, G]),
                                    in1=ior[:, 0:G], op=OP.is_equal)
            nc.tensor.matmul(pool_ps[:], lhsT=og[:], rhs=o2[:],
                             start=(w == 0), stop=(w == WPC - 1))

        edge_layer(t2full[:, :], D2, FOUT, 17, 1, epi2, "b")

        # ---------- pooled allreduce + mean + b2 + log_softmax ----------
        pog = wrk.tile([G, FOUT], F32, tag="pog")
        nc.vector.tensor_copy(pog[:], pool_ps[:])
        nc.sync.dma_start(prloc[:, :], pog[:])
        nc.gpsimd.collective_compute(
            "AllReduce", OP.add,
            replica_groups=[list(range(NCORES))],
            ins=[prloc[:, :]], outs=[prsum[:, :]])
        pk = wrk.tile([G, FOUT], F32, tag="pk")
        nc.sync.dma_start(pk[:], prsum[:, :])
        nc.vector.tensor_tensor(out=pk[:], in0=pk[:],
                                in1=cnr[:, :].to_broadcast([G, FOUT]), op=OP.mult)
        nc.vector.tensor_tensor(out=pk[:], in0=pk[:], in1=b2r[:], op=OP.add)
        mx = wrk.tile([G, 1], F32, tag="mx")
        nc.vector.reduce_max(mx[:], pk[:], axis=AX.X)
        nc.vector.tensor_tensor(out=pk[:], in0=pk[:],
                                in1=mx[:, :].to_broadcast([G, FOUT]), op=OP.subtract)
        exr = wrk.tile([G, FOUT], F32, tag="exr")
        nc.scalar.activation(exr[:], pk[:], AF.Exp)
        sm = wrk.tile([G, 1], F32, tag="sm")
        nc.vector.reduce_sum(sm[:], exr[:], axis=AX.X)
        nc.scalar.activation(sm[:], sm[:], AF.Ln)
        nc.vector.tensor_tensor(out=pk[:], in0=pk[:],
                                in1=sm[:, :].to_broadcast([G, FOUT]), op=OP.subtract)
        nc.sync.dma_start(p_out[:, :], pk[:])

    nc.compile()
    return nc


_CACHE = {}


def kernel(x, edge_index, batch, W1, att1_src, att1_dst, b1, W2, att2_src, att2_dst, b2,
           _trace=False):
    consts, percore, tiles_per_lw, tile_base, T = _prep(
        np.asarray(x), np.asarray(edge_index), np.asarray(batch),
        np.asarray(W1, np.float32), np.asarray(att1_src, np.float32),
        np.asarray(att1_dst, np.float32), np.asarray(b1, np.float32),
        np.asarray(W2, np.float32), np.asarray(att2_src, np.float32),
        np.asarray(att2_dst, np.float32), np.asarray(b2, np.float32))

    key = ("k", NCORES, T, tuple(int(v) for v in tiles_per_lw))
    if key not in _CACHE:
        _CACHE[key] = _build(tiles_per_lw, tile_base, T)
    nc = _CACHE[key]

    in_maps = []
    for c in range(NCORES):
        m = dict(consts)
        m.update(percore[c])
        in_maps.append(m)

    from concourse.bass_utils import run_bass_kernel_spmd
    res = run_bass_kernel_spmd(nc, in_maps, core_ids=list(range(NCORES)),
                               trace=_trace)
    if _trace:
        print(f"HW exec time: {res.exec_time_ns} ns")
    return res.results[0]["out"].astype(np.float32)

